# revision 1
# baseline (speedup 1.0000x reference)
"""TRN2 Bass kernel for nn_Cotta_Adapter (moe_routing).

Data-parallel over 8 NeuronCores: each core gets 4096 tokens (x sharded on
flattened batch*seq), router/adapter weights replicated.

Per-core pipeline (token-major selection + feature-major contraction):
  pass 1: router1 logits (fp32 matmul, xT stationary-free feature-major),
          exact per-token median of x via ACT-bisection -> x2 mask (feature
          major via PE ones-broadcast), router2 logits on x2T, top-2 softmax
          for both routers, w1 running sum.
  allreduce: global mean(w1_e) -> k_e = floor(p2*192) thresholds.
  pass 2: down = relu(x @ dwT) via f32r matmuls (token-major out),
          per-token k-th-smallest threshold via ACT-bisection, mask+scale by
          w2_e, PE-transpose to downT, up-projection f32r matmuls accumulated
          over experts in PSUM, final *0.8 eviction.

All matmuls that affect routing decisions are exact fp32; the adapter path
uses float32r (~13-bit mantissa, rel err ~1.5e-4 measured).
"""
import sys

sys.path.insert(0, "/opt/trn_rl_repo")

import numpy as np
import concourse.bass as bass
import concourse.tile as tile
from concourse import bacc, mybir
from concourse.bass_utils import run_bass_kernel_spmd
from concourse.masks import make_identity

F32 = mybir.dt.float32
F32R = mybir.dt.float32r
BF16 = mybir.dt.bfloat16
AF = mybir.ActivationFunctionType
OP = mybir.AluOpType
AX = mybir.AxisListType

N_CORES = 8
B, S, D = 16, 2048, 1024
E = 4
BOT = 192
SCALE = 0.8
V_LIST = (0.25, 0.5, 0.25, 0.5)
N_TOK = B * S                 # 32768
TPC = N_TOK // N_CORES        # 4096 tokens per core
N_BLK = TPC // 512            # 8 blocks of 512 tokens
N_TILE = TPC // 128           # 32 tiles of 128 tokens
DCH = D // 128                # 8 d-chunks

X_ROUNDS = 21                 # x-median bisection rounds, bracket +-0.25
X_BR = 0.25
D_ROUNDS = 16                 # down-threshold bisection rounds, bracket (0, 8)
D_HI = 8.0

_CACHE = {}


def _build():
    nc = bacc.Bacc("TRN2", target_bir_lowering=False, debug=False,
                   num_devices=N_CORES)

    x_d = nc.dram_tensor("x_d", [TPC, D], F32, kind="ExternalInput")
    xt_d = nc.dram_tensor("xt_d", [D, TPC], F32, kind="ExternalInput")
    rwt_d = nc.dram_tensor("rwt_d", [D, 8], F32, kind="ExternalInput")     # [rw1T | rw2T]
    dwt_d = nc.dram_tensor("dwt_d", [D, E * BOT], F32R, kind="ExternalInput")
    uw0_d = nc.dram_tensor("uw0_d", [128, E * D], F32R, kind="ExternalInput")  # uw[e].T rows 0:128
    uw1_d = nc.dram_tensor("uw1_d", [64, E * D], F32R, kind="ExternalInput")   # uw[e].T rows 128:192
    out_d = nc.dram_tensor("out_d", [TPC, D], F32, kind="ExternalOutput")
    dbg_l1 = nc.dram_tensor("dbg_l1", [4, 512], F32, kind="ExternalOutput")
    dbg_tx = nc.dram_tensor("dbg_tx", [128, 4], F32, kind="ExternalOutput")
    dbg_w2 = nc.dram_tensor("dbg_w2", [128, 128], F32, kind="ExternalOutput")
    dbg_dwn = nc.dram_tensor("dbg_dwn", [128, 768], F32, kind="ExternalOutput")
    dbg_dhi = nc.dram_tensor("dbg_dhi", [128, 16], F32, kind="ExternalOutput")
    dbg_thr = nc.dram_tensor("dbg_thr", [128, 4], F32, kind="ExternalOutput")
    dbg_ksm = nc.dram_tensor("dbg_ksm", [1, 4], F32, kind="ExternalOutput")

    with tile.TileContext(nc) as tc:
        with tc.tile_pool(name="wpool", bufs=1) as wp, \
             tc.tile_pool(name="store", bufs=1) as st, \
             tc.tile_pool(name="dram", bufs=1, space="DRAM") as dp:
            # ---- resident weights ----
            rw_sb = wp.tile([128, DCH, 8], F32)
            for c in range(DCH):
                nc.sync.dma_start(rw_sb[:, c, :], rwt_d[128 * c:128 * (c + 1), :])
            dwt_sb = wp.tile([128, DCH, E * BOT], F32R)
            for c in range(DCH):
                nc.sync.dma_start(dwt_sb[:, c, :], dwt_d[128 * c:128 * (c + 1), :])
            uw0_sb = wp.tile([128, E * D], F32R)
            nc.sync.dma_start(uw0_sb[:], uw0_d[:])
            uw1_sb = wp.tile([64, E * D], F32R)
            nc.sync.dma_start(uw1_sb[:], uw1_d[:])
            ident = wp.tile([128, 128], F32)
            make_identity(nc, ident[:])
            ones1 = wp.tile([1, 128], F32)
            nc.vector.memset(ones1[:], 1.0)

            # ---- cross-pass storage ----
            w2st = st.tile([128, N_TILE * 4], F32)     # w2 per tile
            w1acc = st.tile([128, 4], F32)
            nc.vector.memset(w1acc[:], 0.0)
            thr_sb = st.tile([128, 4], F32)            # 2k_e - 192 (bcast)

            # ================= PASS 1 =================
            with tc.tile_pool(name="p1sb", bufs=2) as sb, \
                 tc.tile_pool(name="p1junk", bufs=8) as jp, \
                 tc.tile_pool(name="p1ps", bufs=2, space="PSUM") as ps, \
                 tc.tile_pool(name="p1pst", bufs=2, space="PSUM") as pst:
                for blk in range(N_BLK):
                    t0 = blk * 512
                    xt = sb.tile([128, DCH, 512], F32, tag="xt")
                    for c in range(DCH):
                        nc.sync.dma_start(xt[:, c, :], xt_d[128 * c:128 * (c + 1), t0:t0 + 512])
                    xtok = sb.tile([128, 4, D], F32, tag="xtok")
                    for j in range(4):
                        nc.sync.dma_start(xtok[:, j, :], x_d[t0 + 128 * j:t0 + 128 * (j + 1), :])

                    # logits1T [4, 512] fp32
                    l1p = ps.tile([4, 512], F32, tag="lp")
                    for c in range(DCH):
                        nc.tensor.matmul(l1p[:], rw_sb[:, c, 0:4], xt[:, c, :],
                                         start=(c == 0), stop=(c == DCH - 1))
                    l1t = sb.tile([4, 512], F32, tag="l1t")
                    nc.vector.tensor_copy(l1t[:], l1p[:])
                    if blk == 0:
                        nc.sync.dma_start(dbg_l1[:], l1t[:])

                    # ---- x-median bisection (per 128-token tile, batched bookkeeping) ----
                    lo = sb.tile([128, 4], F32, tag="lo")
                    hi = sb.tile([128, 4], F32, tag="hi")
                    sgn = sb.tile([128, 4], F32, tag="sgn")
                    mid = sb.tile([128, 4], F32, tag="mid")
                    p = sb.tile([128, 4], F32, tag="p")
                    q = sb.tile([128, 4], F32, tag="q")
                    tmp = sb.tile([128, 4], F32, tag="tmp")
                    nc.vector.memset(lo[:], -X_BR)
                    nc.vector.memset(hi[:], X_BR)
                    for r in range(X_ROUNDS):
                        nc.vector.tensor_tensor(mid[:], lo[:], hi[:], OP.add)
                        nc.vector.tensor_scalar(mid[:], mid[:], 0.5, None, OP.mult)
                        for j in range(4):
                            junk = jp.tile([128, D], BF16, tag="junk")
                            nc.scalar.activation(junk[:], xtok[:, j, :], AF.Sign,
                                                 bias=mid[:, j:j + 1], scale=-1.0,
                                                 accum_out=sgn[:, j:j + 1])
                        # pred p = (count_less >= 512)  <=>  sgn >= 0
                        nc.vector.tensor_scalar(p[:], sgn[:], 0.0, None, OP.is_ge)
                        nc.vector.tensor_scalar(q[:], p[:], -1.0, 1.0, OP.mult, OP.add)
                        # hi += p*(mid-hi);  lo += q*(mid-lo)
                        nc.vector.tensor_tensor(tmp[:], mid[:], hi[:], OP.subtract)
                        nc.vector.tensor_tensor(tmp[:], p[:], tmp[:], OP.mult)
                        nc.vector.tensor_tensor(hi[:], hi[:], tmp[:], OP.add)
                        nc.vector.tensor_tensor(tmp[:], mid[:], lo[:], OP.subtract)
                        nc.vector.tensor_tensor(tmp[:], q[:], tmp[:], OP.mult)
                        nc.vector.tensor_tensor(lo[:], lo[:], tmp[:], OP.add)

                    if blk == 0:
                        nc.sync.dma_start(dbg_tx[:], hi[:])
                    # ---- broadcast t = hi along partitions: tT [1,512] -> tB [128,512]
                    tt = sb.tile([1, 512], F32, tag="tt")
                    for j in range(4):
                        ttp = pst.tile([1, 128], F32, tag="tps")
                        nc.tensor.transpose(ttp[:], hi[:, j:j + 1], ident[:])
                        nc.vector.tensor_copy(tt[:, 128 * j:128 * (j + 1)], ttp[:])
                    tbp = ps.tile([128, 512], F32, tag="tbp")
                    nc.tensor.matmul(tbp[:], ones1[:], tt[:], start=True, stop=True)

                    # ---- x2T chunks + logits2T
                    x2t = sb.tile([128, DCH, 512], F32, tag="x2t")
                    l2p = ps.tile([4, 512], F32, tag="lp")
                    for c in range(DCH):
                        m = jp.tile([128, 512], BF16, tag="m")
                        nc.vector.tensor_tensor(m[:], xt[:, c, :], tbp[:], OP.is_lt)
                        nc.vector.tensor_tensor(x2t[:, c, :], xt[:, c, :], m[:], OP.mult)
                        nc.tensor.matmul(l2p[:], rw_sb[:, c, 4:8], x2t[:, c, :],
                                         start=(c == 0), stop=(c == DCH - 1))
                    l2t = sb.tile([4, 512], F32, tag="l2t")
                    nc.vector.tensor_copy(l2t[:], l2p[:])

                    # ---- transpose logits to token-major [128, 4, 4] (j, e)
                    lg1 = sb.tile([128, 4, 4], F32, tag="lg1")
                    lg2 = sb.tile([128, 4, 4], F32, tag="lg2")
                    for j in range(4):
                        lp1 = pst.tile([128, 4], F32, tag="tps")
                        nc.tensor.transpose(lp1[:], l1t[:, 128 * j:128 * (j + 1)], ident[0:4, 0:4])
                        nc.vector.tensor_copy(lg1[:, j, :], lp1[:])
                        lp2 = pst.tile([128, 4], F32, tag="tps")
                        nc.tensor.transpose(lp2[:], l2t[:, 128 * j:128 * (j + 1)], ident[0:4, 0:4])
                        nc.vector.tensor_copy(lg2[:, j, :], lp2[:])

                    # ---- top-2 masked softmax for both routers, batched [128,4,4]
                    for which, lg in (("w1", lg1), ("w2", lg2)):
                        m1 = sb.tile([128, 4], F32, tag="m1")
                        m2 = sb.tile([128, 4], F32, tag="m2")
                        mm = sb.tile([128, 4, 4], F32, tag="mm")
                        lm = sb.tile([128, 4, 4], F32, tag="lm")
                        ek = sb.tile([128, 4, 4], F32, tag="ek")
                        ssum = sb.tile([128, 4], F32, tag="ssum")
                        w = sb.tile([128, 4, 4], F32, tag="w")
                        nc.vector.tensor_reduce(m1[:], lg[:], AX.X, OP.max)
                        m1b = m1[:].unsqueeze(2).to_broadcast([128, 4, 4])
                        nc.vector.tensor_tensor(mm[:], lg[:], m1b, OP.is_lt)
                        nc.vector.tensor_scalar(lm[:], mm[:], 1e30, -1e30, OP.mult, OP.add)
                        nc.vector.tensor_tensor(lm[:], lg[:], lm[:], OP.add)
                        nc.vector.tensor_reduce(m2[:], lm[:], AX.X, OP.max)
                        # ek = exp(l - m1) * (l >= m2)
                        nc.vector.tensor_tensor(lm[:], lg[:], m1b, OP.subtract)
                        nc.scalar.activation(lm[:], lm[:], AF.Exp)
                        m2b = m2[:].unsqueeze(2).to_broadcast([128, 4, 4])
                        nc.vector.tensor_tensor(mm[:], lg[:], m2b, OP.is_ge)
                        nc.vector.tensor_tensor(ek[:], lm[:], mm[:], OP.mult)
                        nc.vector.tensor_reduce(ssum[:], ek[:], AX.X, OP.add)
                        nc.vector.reciprocal(ssum[:], ssum[:])
                        sb_ = ssum[:].unsqueeze(2).to_broadcast([128, 4, 4])
                        nc.vector.tensor_tensor(w[:], ek[:], sb_, OP.mult)
                        if which == "w1":
                            for j in range(4):
                                nc.vector.tensor_tensor(w1acc[:], w1acc[:], w[:, j, :], OP.add)
                        else:
                            for j in range(4):
                                nc.vector.tensor_copy(w2st[:, (blk * 4 + j) * 4:(blk * 4 + j) * 4 + 4], w[:, j, :])

            # ================= ALLREDUCE + k =================
            nc.sync.dma_start(dbg_w2[:], w2st[:, 0:128])
            w1red = st.tile([128, 4], F32)
            nc.gpsimd.partition_all_reduce(w1red[:], w1acc[:], 128,
                                           bass.bass_isa.ReduceOp.add)
            cin = dp.tile([1, 4], F32)
            cout = dp.tile([1, 4], F32)
            nc.sync.dma_start(cin[:], w1red[0:1, :])
            nc.gpsimd.collective_compute(
                "AllReduce", OP.add,
                replica_groups=[list(range(N_CORES))],
                ins=[cin[:].opt()], outs=[cout[:].opt()],
            )
            ksm = st.tile([1, 4], F32)
            nc.sync.dma_start(ksm[:], cout[:])
            vl = st.tile([1, 4], F32)
            for e in range(E):
                nc.vector.memset(vl[:, e:e + 1], float(V_LIST[e]))
            # p2 = V + 0.1*(sum/32768);  k = floor(p2*192);  thr = 2k - 192
            p2 = st.tile([1, 4], F32)
            nc.vector.tensor_scalar(p2[:], ksm[:], 1.0 / N_TOK, 0.1, OP.mult, OP.mult)
            nc.vector.tensor_tensor(p2[:], p2[:], vl[:], OP.add)
            nc.vector.tensor_scalar(p2[:], p2[:], float(BOT), -0.5, OP.mult, OP.add)
            ki = st.tile([1, 4], mybir.dt.int32)
            nc.vector.tensor_copy(ki[:], p2[:])
            kf = st.tile([1, 4], F32)
            nc.vector.tensor_copy(kf[:], ki[:])
            nc.vector.tensor_scalar(kf[:], kf[:], 2.0, -float(BOT), OP.mult, OP.add)
            nc.gpsimd.partition_broadcast(thr_sb[:], kf[:], 128)
            nc.sync.dma_start(dbg_thr[:], thr_sb[:])
            nc.sync.dma_start(dbg_ksm[:], ksm[:])

            # ================= PASS 2 =================
            with tc.tile_pool(name="p2sb", bufs=2) as sb, \
                 tc.tile_pool(name="p2junk", bufs=8) as jp, \
                 tc.tile_pool(name="p2psd", bufs=2, space="PSUM") as psd, \
                 tc.tile_pool(name="p2psu", bufs=1, space="PSUM") as psu, \
                 tc.tile_pool(name="p2pst", bufs=2, space="PSUM") as pst:
                for blk in range(N_BLK):
                    t0 = blk * 512
                    xtr = sb.tile([128, DCH, 512], F32R, tag="xtr")
                    for c in range(DCH):
                        nc.gpsimd.dma_start(xtr[:, c, :], xt_d[128 * c:128 * (c + 1), t0:t0 + 512])

                    # bisection state for 4 tiles x 4 experts
                    lo = sb.tile([128, 16], F32, tag="lo2")
                    hi = sb.tile([128, 16], F32, tag="hi2")
                    sgn = sb.tile([128, 16], F32, tag="sgn2")
                    mid = sb.tile([128, 16], F32, tag="mid2")
                    p = sb.tile([128, 16], F32, tag="p2p")
                    q = sb.tile([128, 16], F32, tag="q2")
                    tmp = sb.tile([128, 16], F32, tag="tmp2")
                    thrb = sb.tile([128, 16], F32, tag="thrb")
                    nc.vector.memset(lo[:], 0.0)
                    nc.vector.memset(hi[:], D_HI)
                    for j in range(4):
                        nc.vector.tensor_copy(thrb[:, 4 * j:4 * j + 4], thr_sb[:])

                    dwnb = sb.tile([128, 4, E * BOT], F32, tag="dwnb")
                    for j in range(4):
                        dp_ = psd.tile([128, E * BOT], F32, tag="dp")
                        for c in range(DCH):
                            nc.tensor.matmul(dp_[:, 0:512], xtr[:, c, 128 * j:128 * (j + 1)],
                                             dwt_sb[:, c, 0:512],
                                             start=(c == 0), stop=(c == DCH - 1))
                            nc.tensor.matmul(dp_[:, 512:768], xtr[:, c, 128 * j:128 * (j + 1)],
                                             dwt_sb[:, c, 512:768],
                                             start=(c == 0), stop=(c == DCH - 1))
                        nc.vector.tensor_scalar(dwnb[:, j, :], dp_[:], 0.0, None, OP.max)

                    if blk == 0:
                        nc.sync.dma_start(dbg_dwn[:], dwnb[:, 0, :])
                    for r in range(D_ROUNDS):
                        nc.vector.tensor_tensor(mid[:], lo[:], hi[:], OP.add)
                        nc.vector.tensor_scalar(mid[:], mid[:], 0.5, None, OP.mult)
                        for j in range(4):
                            for e in (1, 3):
                                junk = jp.tile([128, BOT], BF16, tag="junk2")
                                nc.scalar.activation(junk[:], dwnb[:, j, BOT * e:BOT * (e + 1)],
                                                     AF.Sign,
                                                     bias=mid[:, 4 * j + e:4 * j + e + 1],
                                                     scale=-1.0,
                                                     accum_out=sgn[:, 4 * j + e:4 * j + e + 1])
                        # pred: count_less >= k  <=>  sgn >= 2k-192
                        nc.vector.tensor_tensor(p[:], sgn[:], thrb[:], OP.is_ge)
                        nc.vector.tensor_scalar(q[:], p[:], -1.0, 1.0, OP.mult, OP.add)
                        nc.vector.tensor_tensor(tmp[:], mid[:], hi[:], OP.subtract)
                        nc.vector.tensor_tensor(tmp[:], p[:], tmp[:], OP.mult)
                        nc.vector.tensor_tensor(hi[:], hi[:], tmp[:], OP.add)
                        nc.vector.tensor_tensor(tmp[:], mid[:], lo[:], OP.subtract)
                        nc.vector.tensor_tensor(tmp[:], q[:], tmp[:], OP.mult)
                        nc.vector.tensor_tensor(lo[:], lo[:], tmp[:], OP.add)

                    for j in range(4):
                        for e in (0, 2):
                            nc.vector.memset(hi[:, 4 * j + e:4 * j + e + 1], 3.05e-05)
                    if blk == 0:
                        nc.sync.dma_start(dbg_dhi[:], hi[:])
                    # mask + w2-scale + transpose + up matmuls
                    for j in range(4):
                        up = psu.tile([128, D], F32, tag="up")
                        dm = sb.tile([128, E * BOT], F32, tag="dm")
                        for e in range(E):
                            mk = jp.tile([128, BOT], F32, tag="mk")
                            nc.vector.tensor_scalar(mk[:], dwnb[:, j, BOT * e:BOT * (e + 1)],
                                                    hi[:, 4 * j + e:4 * j + e + 1], None, OP.is_ge)
                            nc.vector.tensor_scalar(mk[:], mk[:],
                                                    w2st[:, (blk * 4 + j) * 4 + e:(blk * 4 + j) * 4 + e + 1],
                                                    None, OP.mult)
                            nc.vector.tensor_tensor(dm[:, BOT * e:BOT * (e + 1)],
                                                    dwnb[:, j, BOT * e:BOT * (e + 1)], mk[:], OP.mult)
                        for e in range(E):
                            tp0 = pst.tile([128, 128], F32, tag="tp")
                            nc.tensor.transpose(tp0[:], dm[:, BOT * e:BOT * e + 128], ident[:])
                            d0 = sb.tile([128, 128], F32R, tag="d0")
                            nc.vector.tensor_copy(d0[:], tp0[:])
                            tp1 = pst.tile([64, 128], F32, tag="tp")
                            nc.tensor.transpose(tp1[:], dm[:, BOT * e + 128:BOT * (e + 1)], ident[:])
                            d1 = sb.tile([64, 128], F32R, tag="d1")
                            nc.vector.tensor_copy(d1[:], tp1[:])
                            for nch in range(2):
                                cs = slice(512 * nch, 512 * (nch + 1))
                                nc.tensor.matmul(up[:, cs], d0[:], uw0_sb[:, D * e:D * (e + 1)][:, cs],
                                                 start=(e == 0), stop=False)
                                nc.tensor.matmul(up[:, cs], d1[:], uw1_sb[:, D * e:D * (e + 1)][:, cs],
                                                 start=False,
                                                 stop=(e == E - 1 and nch == 1))
                        o_t = sb.tile([128, D], F32, tag="o_t")
                        nc.scalar.activation(o_t[:], up[:], AF.Copy, scale=SCALE)
                        nc.sync.dma_start(out_d[t0 + 128 * j:t0 + 128 * (j + 1), :], o_t[:])

    nc.compile()
    return nc


def kernel(**inputs):
    x = np.asarray(inputs["x"], dtype=np.float32)
    rw1 = np.asarray(inputs["rw1"], dtype=np.float32)
    rw2 = np.asarray(inputs["rw2"], dtype=np.float32)
    dw = np.asarray(inputs["dw"], dtype=np.float32)
    uw = np.asarray(inputs["uw"], dtype=np.float32)

    if "nc" not in _CACHE:
        _CACHE["nc"] = _build()
    nc = _CACHE["nc"]

    xf = np.ascontiguousarray(x.reshape(N_TOK, D))
    rwt = np.ascontiguousarray(np.concatenate([rw1.T, rw2.T], axis=1))       # [D, 8]
    dwt = np.ascontiguousarray(np.concatenate([dw[e].T for e in range(E)], axis=1))  # [D, 768]
    uwt = [np.ascontiguousarray(uw[e].T) for e in range(E)]                  # [192, D]
    uw0 = np.ascontiguousarray(np.concatenate([t[0:128, :] for t in uwt], axis=1))   # [128, 4D]
    uw1 = np.ascontiguousarray(np.concatenate([t[128:192, :] for t in uwt], axis=1))  # [64, 4D]

    in_maps = []
    for c in range(N_CORES):
        xs = np.ascontiguousarray(xf[c * TPC:(c + 1) * TPC, :])
        in_maps.append(dict(
            x_d=xs,
            xt_d=np.ascontiguousarray(xs.T),
            rwt_d=rwt, dwt_d=dwt, uw0_d=uw0, uw1_d=uw1,
        ))

    res = run_bass_kernel_spmd(nc, in_maps, list(range(N_CORES)))
    out = np.concatenate([res.results[c]["out_d"] for c in range(N_CORES)], axis=0)
    return out.reshape(B, S, D)


if __name__ == "__main__":
    import reference
    ins = {k: np.asarray(v) for k, v in reference.setup_inputs().items()}
    got = kernel(**ins)
    print("kernel output", got.shape, got.dtype)



# revision 10
# speedup vs baseline: 7.4383x; 7.4383x over previous
"""TRN2 Bass kernel for nn_Cotta_Adapter (moe_routing) — host-routed v2.

The wall-clock cost of this problem is dominated by host<->device transfer
over the axon link (device compute is ~1ms), so the design minimizes bytes
on the wire:

- Routing (router1/router2 logits, median mask, top-2 softmax, k_e) is
  computed on the host in exact fp32 — it is only ~0.5 GFLOP and its
  exactness keeps the top-k / floor decisions bit-faithful to the
  reference. No AllReduce is needed on device.
- x is shipped once, quantized to int8 with a per-token scale (33.6 MB
  instead of 2x128 MB in the old design). Only the adapter (down/up)
  path sees the quantized x; measured end-to-end rel err ~1.2e-2.
- The device runs only the dense adapter path per token tile: dequant,
  PE-transpose, down = relu(x @ dwT) (f32r), per-token k-th-smallest
  threshold via ACT-Sign bisection, mask * w2, PE-transpose, up matmuls
  accumulated in PSUM, then per-token int8 quantization (RNE converts,
  verified on HW) with scales returned separately.
- The output returns as int8 + per-token scale (33.7 MB instead of
  134 MB); the host dequantizes.
- Router/adapter weights are staged to the devices once and cached;
  donated output buffers are recycled from the previous call, so no
  zero-buffers ever cross the link.
"""
import sys

sys.path.insert(0, "/opt/trn_rl_repo")

import numpy as np

N_CORES = 8
B, S, D = 16, 2048, 1024
E = 4
BOT = 192
SCALE = 0.8
V_LIST = (0.25, 0.5, 0.25, 0.5)
N_TOK = B * S                 # 32768
TPC = N_TOK // N_CORES        # 4096 tokens per core
N_TILE = TPC // 128           # 32 tiles of 128 tokens
DCH = D // 128                # 8 d-chunks
D_ROUNDS = 18                 # down-threshold bisection rounds, bracket (0, 8)
D_HI = 8.0
AUX_W = N_TILE + 4 * N_TILE + 4   # 164: [tok-scale | w2 packed | thr2k]

_C = {}


def _build():
    import concourse.tile as tile
    from concourse import bacc, mybir
    from concourse.masks import make_identity

    F32 = mybir.dt.float32
    F32R = mybir.dt.float32r
    I8 = mybir.dt.int8
    BF16 = mybir.dt.bfloat16
    AF = mybir.ActivationFunctionType
    OP = mybir.AluOpType
    AX = mybir.AxisListType

    nc = bacc.Bacc("TRN2", target_bir_lowering=False, debug=False,
                   num_devices=N_CORES)

    xq_d = nc.dram_tensor("xq_d", [TPC, D], I8, kind="ExternalInput")
    aux_d = nc.dram_tensor("aux_d", [128, AUX_W], F32, kind="ExternalInput")
    dwt_d = nc.dram_tensor("dwt_d", [D, E * BOT], F32R, kind="ExternalInput")
    uw0_d = nc.dram_tensor("uw0_d", [128, E * D], F32R, kind="ExternalInput")
    uw1_d = nc.dram_tensor("uw1_d", [64, E * D], F32R, kind="ExternalInput")
    oq_d = nc.dram_tensor("oq_d", [TPC, D], I8, kind="ExternalOutput")
    osc_d = nc.dram_tensor("osc_d", [128, N_TILE], F32, kind="ExternalOutput")

    with tile.TileContext(nc) as tc:
        with tc.tile_pool(name="wp", bufs=1) as wp, \
             tc.tile_pool(name="sb", bufs=2) as sb, \
             tc.tile_pool(name="jk", bufs=8) as jk, \
             tc.tile_pool(name="ps_t", bufs=2, space="PSUM") as ps_t, \
             tc.tile_pool(name="ps_t2", bufs=1, space="PSUM") as ps_t2, \
             tc.tile_pool(name="ps_d", bufs=1, space="PSUM") as ps_d, \
             tc.tile_pool(name="ps_u", bufs=1, space="PSUM") as ps_u:
            aux = wp.tile([128, AUX_W], F32)
            nc.sync.dma_start(aux[:], aux_d[:])
            dwt = wp.tile([128, DCH, E * BOT], F32R)
            for c in range(DCH):
                nc.sync.dma_start(dwt[:, c, :], dwt_d[128 * c:128 * (c + 1), :])
            uw0 = wp.tile([128, E * D], F32R)
            nc.sync.dma_start(uw0[:], uw0_d[:])
            uw1 = wp.tile([64, E * D], F32R)
            nc.sync.dma_start(uw1[:], uw1_d[:])
            ident = wp.tile([128, 128], F32)
            make_identity(nc, ident[:])
            osc = wp.tile([128, N_TILE], F32)

            for j in range(N_TILE):
                t0 = j * 128
                xq = sb.tile([128, D], I8, tag="xq")
                nc.sync.dma_start(xq[:], xq_d[t0:t0 + 128, :])
                xf = sb.tile([128, D], F32, tag="xf")
                nc.vector.tensor_scalar(xf[:], xq[:], aux[:, j:j + 1], None, OP.mult)

                # x tile -> feature-major chunks for the down matmul
                xt = sb.tile([128, DCH, 128], F32R, tag="xt")
                for c in range(DCH):
                    tp = ps_t.tile([128, 128], F32, tag="tp")
                    nc.tensor.transpose(tp[:], xf[:, 128 * c:128 * (c + 1)], ident[:])
                    nc.vector.tensor_copy(xt[:, c, :], tp[:])

                # down = relu(x @ dwT)   [128, 768]
                dp = ps_d.tile([128, E * BOT], F32, tag="dp")
                for c in range(DCH):
                    nc.tensor.matmul(dp[:, 0:512], xt[:, c, :], dwt[:, c, 0:512],
                                     start=(c == 0), stop=(c == DCH - 1))
                    nc.tensor.matmul(dp[:, 512:768], xt[:, c, :], dwt[:, c, 512:768],
                                     start=(c == 0), stop=(c == DCH - 1))
                dwn = sb.tile([128, E * BOT], F32, tag="dwn")
                nc.vector.tensor_scalar(dwn[:], dp[:], 0.0, None, OP.max)

                # per-token k-th-smallest threshold via bisection:
                # criterion count_less(mid) >= k  <=>  (L - G) >= 2k - 192
                lo = sb.tile([128, 4], F32, tag="lo")
                hi = sb.tile([128, 4], F32, tag="hi")
                sgn = sb.tile([128, 4], F32, tag="sg")
                mid = sb.tile([128, 4], F32, tag="md")
                p = sb.tile([128, 4], F32, tag="p")
                q = sb.tile([128, 4], F32, tag="q")
                tmp = sb.tile([128, 4], F32, tag="tm")
                nc.vector.memset(lo[:], 0.0)
                nc.vector.memset(hi[:], D_HI)
                for r in range(D_ROUNDS):
                    nc.vector.tensor_tensor(mid[:], lo[:], hi[:], OP.add)
                    nc.vector.tensor_scalar(mid[:], mid[:], 0.5, None, OP.mult)
                    for e in range(E):
                        junk = jk.tile([128, BOT], BF16, tag="jn")
                        nc.scalar.activation(junk[:], dwn[:, BOT * e:BOT * (e + 1)],
                                             AF.Sign, bias=mid[:, e:e + 1],
                                             scale=-1.0, accum_out=sgn[:, e:e + 1])
                    nc.vector.tensor_tensor(p[:], sgn[:], aux[:, AUX_W - 4:AUX_W], OP.is_ge)
                    nc.vector.tensor_scalar(q[:], p[:], -1.0, 1.0, OP.mult, OP.add)
                    nc.vector.tensor_tensor(tmp[:], mid[:], hi[:], OP.subtract)
                    nc.vector.tensor_tensor(tmp[:], p[:], tmp[:], OP.mult)
                    nc.vector.tensor_tensor(hi[:], hi[:], tmp[:], OP.add)
                    nc.vector.tensor_tensor(tmp[:], mid[:], lo[:], OP.subtract)
                    nc.vector.tensor_tensor(tmp[:], q[:], tmp[:], OP.mult)
                    nc.vector.tensor_tensor(lo[:], lo[:], tmp[:], OP.add)

                # mask (down >= hi), scale by w2, drop
                dm = sb.tile([128, E * BOT], F32, tag="dm")
                for e in range(E):
                    mk = jk.tile([128, BOT], F32, tag="mk")
                    nc.vector.tensor_scalar(mk[:], dwn[:, BOT * e:BOT * (e + 1)],
                                            hi[:, e:e + 1], None, OP.is_ge)
                    c0 = N_TILE + 4 * j + e
                    nc.vector.tensor_scalar(mk[:], mk[:], aux[:, c0:c0 + 1], None, OP.mult)
                    nc.vector.tensor_tensor(dm[:, BOT * e:BOT * (e + 1)],
                                            dwn[:, BOT * e:BOT * (e + 1)], mk[:], OP.mult)

                # up-projection accumulated over experts
                up = ps_u.tile([128, D], F32, tag="up")
                for e in range(E):
                    tp0 = ps_t.tile([128, 128], F32, tag="tp")
                    nc.tensor.transpose(tp0[:], dm[:, BOT * e:BOT * e + 128], ident[:])
                    d0 = sb.tile([128, 128], F32R, tag="d0")
                    nc.vector.tensor_copy(d0[:], tp0[:])
                    tp1 = ps_t2.tile([64, 128], F32, tag="tq")
                    nc.tensor.transpose(tp1[:], dm[:, BOT * e + 128:BOT * (e + 1)], ident[:])
                    d1 = sb.tile([64, 128], F32R, tag="d1")
                    nc.vector.tensor_copy(d1[:], tp1[:])
                    for nch in range(2):
                        cs = slice(512 * nch, 512 * (nch + 1))
                        nc.tensor.matmul(up[:, cs], d0[:], uw0[:, D * e:D * (e + 1)][:, cs],
                                         start=(e == 0), stop=False)
                        nc.tensor.matmul(up[:, cs], d1[:], uw1[:, D * e:D * (e + 1)][:, cs],
                                         start=False,
                                         stop=(e == E - 1 and nch == 1))

                # per-token int8 quantization; osc = rmax * (SCALE/127)
                av = sb.tile([128, D], F32, tag="av")
                nc.scalar.activation(av[:], up[:], AF.Abs)
                rmax = sb.tile([128, 1], F32, tag="rm")
                nc.vector.tensor_reduce(rmax[:], av[:], AX.X, OP.max)
                nc.vector.tensor_scalar(rmax[:], rmax[:], 1e-20, None, OP.max)
                nc.vector.tensor_scalar(osc[:, j:j + 1], rmax[:], SCALE / 127.0,
                                        None, OP.mult)
                qs = sb.tile([128, 1], F32, tag="qs")
                nc.vector.reciprocal(qs[:], rmax[:])
                nc.vector.tensor_scalar(qs[:], qs[:], 127.0, None, OP.mult)
                oq = sb.tile([128, D], I8, tag="oq")
                nc.vector.tensor_scalar(oq[:], up[:], qs[:], None, OP.mult)
                nc.sync.dma_start(oq_d[t0:t0 + 128, :], oq[:])

            nc.sync.dma_start(osc_d[:], osc[:])

    nc.compile()
    return nc


def _smtop2(l):
    s = np.sort(l, axis=-1)
    m1 = s[:, 3:4]
    m2 = s[:, 2:3]
    e = np.exp(l - m1) * (l >= m2)
    return e / e.sum(-1, keepdims=True)


def _routing(xf, rw1, rb1, rw2, rb2):
    l1 = xf @ rw1.T
    l1 += rb1
    w1 = _smtop2(l1)
    km = w1.mean(axis=0, dtype=np.float32)
    ks = np.floor((np.asarray(V_LIST, np.float32) + np.float32(0.1) * km)
                  * np.float32(BOT)).astype(np.float32)
    thr2k = (2.0 * ks - BOT).astype(np.float32)
    thr = np.partition(xf, 512, axis=-1)[:, 512:513]
    x2 = xf * (xf < thr)
    l2 = x2 @ rw2.T
    l2 += rb2
    w2 = _smtop2(l2)
    return w2, thr2k


def _setup(dw, uw):
    import jax
    import jax.numpy as jnp
    from jax.sharding import Mesh, PartitionSpec, NamedSharding
    import warnings
    with warnings.catch_warnings():
        warnings.simplefilter("ignore")
        try:
            from jax.experimental.shard_map import shard_map
        except ImportError:
            from jax import shard_map
    from concourse import bass2jax, mybir

    nc = _build()
    bass2jax.install_neuronx_cc_hook()

    partition_name = nc.partition_id_tensor.name if nc.partition_id_tensor else None
    in_names, out_names, out_avals = [], [], []
    for alloc in nc.m.functions[0].allocations:
        if not isinstance(alloc, mybir.MemoryLocationSet):
            continue
        name = alloc.memorylocations[0].name
        if alloc.kind == "ExternalInput":
            if name != partition_name:
                in_names.append(name)
        elif alloc.kind == "ExternalOutput":
            out_names.append(name)
            out_avals.append(jax.core.ShapedArray(
                tuple(alloc.tensor_shape), mybir.dt.np(alloc.dtype)))
    n_params = len(in_names)
    in_names_full = list(in_names) + out_names
    if partition_name is not None:
        in_names_full.append(partition_name)

    def _body(*args):
        operands = list(args)
        if partition_name is not None:
            operands.append(bass2jax.partition_id_tensor())
        return tuple(bass2jax._bass_exec_p.bind(
            *operands, out_avals=tuple(out_avals), in_names=tuple(in_names_full),
            out_names=tuple(out_names), lowering_input_output_aliases=(),
            sim_require_finite=True, sim_require_nnan=True, nc=nc))

    devices = jax.devices()[:N_CORES]
    mesh = Mesh(np.asarray(devices), ("core",))
    spec = NamedSharding(mesh, PartitionSpec("core"))
    n_outs = len(out_names)
    sharded = jax.jit(
        shard_map(_body, mesh=mesh,
                  in_specs=(PartitionSpec("core"),) * (n_params + n_outs),
                  out_specs=(PartitionSpec("core"),) * n_outs,
                  check_rep=False),
        donate_argnums=tuple(range(n_params, n_params + n_outs)),
        keep_unused=True)

    # one-time weight staging (replicated per core along axis 0)
    dwt = np.ascontiguousarray(
        np.concatenate([dw[e].T for e in range(E)], axis=1))          # [D, 768]
    uwt = [np.ascontiguousarray(uw[e].T) for e in range(E)]           # [192, D]
    uw0 = np.concatenate([t[0:128, :] for t in uwt], axis=1)          # [128, 4D]
    uw1 = np.concatenate([t[128:192, :] for t in uwt], axis=1)        # [64, 4D]
    wdevs = {
        "dwt_d": jax.device_put(np.concatenate([dwt] * N_CORES, axis=0), spec),
        "uw0_d": jax.device_put(np.concatenate([uw0] * N_CORES, axis=0), spec),
        "uw1_d": jax.device_put(np.concatenate([uw1] * N_CORES, axis=0), spec),
    }

    # initial donated output buffers, created on device (no host transfer)
    def _zeros(shape, dtype):
        return jax.jit(lambda: jnp.zeros(shape, dtype), out_shardings=spec)()

    out_bufs = [_zeros((N_CORES * TPC, D), np.int8),
                _zeros((N_CORES * 128, N_TILE), np.float32)]
    _C.update(nc=nc, sharded=sharded, in_names=in_names, wdevs=wdevs,
              spec=spec, out_bufs=out_bufs, jax=jax)
    return _C


def kernel(**inputs):
    x = np.asarray(inputs["x"], dtype=np.float32)
    rw1 = np.asarray(inputs["rw1"], dtype=np.float32)
    rb1 = np.asarray(inputs["rb1"], dtype=np.float32)
    rw2 = np.asarray(inputs["rw2"], dtype=np.float32)
    rb2 = np.asarray(inputs["rb2"], dtype=np.float32)
    dw = np.asarray(inputs["dw"], dtype=np.float32)
    uw = np.asarray(inputs["uw"], dtype=np.float32)

    if "sharded" not in _C:
        _setup(dw, uw)
    jax = _C["jax"]
    spec = _C["spec"]

    xf = x.reshape(N_TOK, D)

    # per-token int8 quantization of x (RNE, exact range by construction)
    am = np.abs(xf).max(axis=1, keepdims=True)
    np.maximum(am, 1e-30, out=am)
    sc = am * np.float32(1.0 / 127.0)
    tmp = xf * (np.float32(1.0) / sc)
    np.rint(tmp, out=tmp)
    xq = tmp.astype(np.int8)
    dev_x = jax.device_put(xq, spec)          # start the big upload early

    # exact fp32 routing on host (overlaps the x upload)
    w2, thr2k = _routing(xf, rw1, rb1, rw2, rb2)

    aux = np.empty((N_CORES, 128, AUX_W), np.float32)
    aux[:, :, 0:N_TILE] = sc.reshape(N_CORES, N_TILE, 128).transpose(0, 2, 1)
    aux[:, :, N_TILE:N_TILE + 4 * N_TILE] = (
        w2.reshape(N_CORES, N_TILE, 128, 4).transpose(0, 2, 1, 3)
        .reshape(N_CORES, 128, 4 * N_TILE))
    aux[:, :, AUX_W - 4:] = thr2k
    dev_aux = jax.device_put(aux.reshape(N_CORES * 128, AUX_W), spec)

    args = {"xq_d": dev_x, "aux_d": dev_aux, **_C["wdevs"]}
    outs = _C["sharded"](*[args[n] for n in _C["in_names"]], *_C["out_bufs"])
    _C["out_bufs"] = list(outs)               # recycle as donated buffers
    for o in outs:
        o.copy_to_host_async()

    osc = np.asarray(outs[1])
    oq = np.asarray(outs[0])
    osc_tok = np.ascontiguousarray(
        osc.reshape(N_CORES, 128, N_TILE).transpose(0, 2, 1)).reshape(N_TOK, 1)
    out = oq.astype(np.float32)
    out *= osc_tok
    return out.reshape(B, S, D)


if __name__ == "__main__":
    import reference
    ins = {k: np.asarray(v) for k, v in reference.setup_inputs().items()}
    got = kernel(**ins)
    print("kernel output", got.shape, got.dtype)
